# revision 10
# baseline (speedup 1.0000x reference)
"""Trainium2 kernel for nn_MAg_90709709292194 (gnn_message_passing).

Computation: out = inputs @ ker_wt + bias, where ker_wt (8192x8192, ~0.9%
dense) holds the `kernel` values scattered into the nonzero pattern of
tile(adjacency, (4, 4)) in row-major nonzero order.

The dense formulation streams 128 MiB of mostly-zero weights; instead this
kernel exploits the graph structure directly. Mirroring the original TF
layer, everything derivable at build() time (adjacency nonzeros, per-edge
4x4 weight blocks, ELL packing/permutations) is host-side prep; the
per-forward-pass math runs on the NeuronCores.

Per-destination-node ELL formulation, dest-sharded over 8 cores (256 dest
nodes per core):
    out[b, co, j] = sum_s sum_ci X[b, ci, src(j, s)] * w[j, s, ci, co]
Each dest node j becomes ONE tensor-engine matmul with K = 128 = (32
in-degree slots x 4 in-channels): stationary = gathered X columns for j's
neighborhood [128, 32 batch] (fp16), moving = that node's packed edge
weights [128, 4 out-channels]. Nodes round-robin the four 32-wide PE column
groups, so four matmuls run concurrently in the array; in-degree > 32
(max 35 here) spills into a second accumulating matmul from a small
overflow block. PSUM accumulates [128 = 4 groups x 32 batch, 256 = 64
nodes x 4 co]; one DVE pass adds bias, and the result is dumped linearly
with the column permutation undone on host.
"""

import numpy as np

N = 2048        # nodes
IC = 4          # input channels
CH = 4          # output channels
B = 32          # batch
NCORES = 8
JPC = N // NCORES   # 256 dest nodes per core
S = 32              # ELL slots (in-degree capacity per matmul)
NXT = 8             # xg streaming tiles (32 nodes each)

_PROGRAM_CACHE = {}


def build_program(ovf, debug=False):
    key = (int(ovf), bool(debug))
    if key in _PROGRAM_CACHE:
        return _PROGRAM_CACHE[key]

    import concourse.bass as bass
    import concourse.bacc as bacc
    import concourse.mybir as mybir
    import concourse.tile as tile

    f32 = mybir.dt.float32
    f16 = mybir.dt.float16

    nc = bacc.Bacc(
        "TRN2", target_bir_lowering=False, debug=debug, num_devices=NCORES
    )
    # xg: gathered neighborhood features, [128=(s,ci), j, b] fp16
    xg_d = nc.dram_tensor("xg", [128, JPC, B], f16, kind="ExternalInput")
    # wm: packed edge weights, [128=(s,ci), j, co] fp16
    wm_d = nc.dram_tensor("wm", [128, JPC, CH], f16, kind="ExternalInput")
    # overflow blocks for nodes with in-degree > S (always >= 1 entry)
    oxg_d = nc.dram_tensor("oxg", [128, ovf, B], f16, kind="ExternalInput")
    owm_d = nc.dram_tensor("owm", [128, ovf, CH], f16, kind="ExternalInput")
    # bias replicated into the physical psum layout [(c,b), (j4,co)] f32
    bias_d = nc.dram_tensor("biasn", [128, JPC], f32, kind="ExternalInput")
    # raw output dump; host undoes the layout permutation
    out_d = nc.dram_tensor("out", [128, JPC], f32, kind="ExternalOutput")

    with tile.TileContext(nc) as tc:
        with (
            tc.tile_pool(name="const", bufs=1) as const,
            tc.tile_pool(name="xgpool", bufs=3) as xgpool,
            tc.tile_pool(name="psum", bufs=1, space=bass.MemorySpace.PSUM) as psum,
        ):
            wm = const.tile([128, JPC * CH], f16)
            nc.scalar.dma_start(out=wm[:, : JPC * CH // 4], in_=wm_d[:, : JPC // 4, :])
            nc.scalar.dma_start(out=wm[:, JPC * CH // 4 :], in_=wm_d[:, JPC // 4 :, :])
            oxg = const.tile([128, ovf * B], f16)
            nc.scalar.dma_start(out=oxg[:], in_=oxg_d[:])
            owm = const.tile([128, ovf * CH], f16)
            nc.scalar.dma_start(out=owm[:], in_=owm_d[:])
            bsn = const.tile([128, JPC], f32)
            nc.scalar.dma_start(out=bsn[:], in_=bias_d[:])

            jpt = JPC // NXT  # nodes per xg tile
            acc = psum.tile([128, JPC], f32)
            for t4 in range(NXT):
                xgt = xgpool.tile([128, jpt * B], f16, tag="xgt")
                nc.sync.dma_start(
                    out=xgt[:], in_=xg_d[:, t4 * jpt : (t4 + 1) * jpt, :]
                )
                for jj in range(jpt):
                    jl = t4 * jpt + jj
                    c = jl % 4
                    j4 = jl // 4
                    nc.tensor.matmul(
                        acc[32 * c : 32 * (c + 1), 4 * j4 : 4 * (j4 + 1)],
                        xgt[:, B * jj : B * (jj + 1)],
                        wm[:, CH * jl : CH * (jl + 1)],
                        start=True,
                        stop=(jl >= ovf),
                        tile_position=(0, 32 * c),
                        skip_group_check=True,
                    )
                    if jl < ovf:
                        # in-degree overflow: accumulate slots S..degmax
                        # immediately so the PSUM group closes right away
                        nc.tensor.matmul(
                            acc[32 * c : 32 * (c + 1), 4 * j4 : 4 * (j4 + 1)],
                            oxg[:, B * jl : B * (jl + 1)],
                            owm[:, CH * jl : CH * (jl + 1)],
                            start=False,
                            stop=True,
                            tile_position=(0, 32 * c),
                            skip_group_check=True,
                        )
            osb = const.tile([128, JPC], f32)
            h = JPC // 2
            nc.vector.tensor_add(osb[:, :h], acc[:, :h], bsn[:, :h])
            nc.sync.dma_start(out=out_d[:, :h], in_=osb[:, :h])
            nc.vector.tensor_add(osb[:, h:], acc[:, h:], bsn[:, h:])
            nc.sync.dma_start(out=out_d[:, h:], in_=osb[:, h:])

    nc.compile()
    _PROGRAM_CACHE[key] = nc
    return nc


def pack_inputs(inputs, adjacency, kernel, bias):
    """Host-side build()-time graph/weight packing + per-core sharding."""
    X = np.asarray(inputs, dtype=np.float32)
    A = np.asarray(adjacency, dtype=np.float32)
    kern = np.asarray(kernel, dtype=np.float32)
    bvec = np.asarray(bias, dtype=np.float32)

    src, dst = np.nonzero(A)          # edge src -> dst, row-major order
    nnz = src.shape[0]
    rnnz = np.bincount(src, minlength=N).astype(np.int64)
    prefix = np.concatenate([[0], np.cumsum(rnnz)[:-1]])
    k_in_row = np.arange(nnz, dtype=np.int64) - prefix[src]
    # per-edge 4x4 weight block, w_e[ci, co]
    wedge = np.empty((nnz, IC, CH), np.float32)
    for ci in range(IC):
        for co in range(CH):
            wedge[:, ci, co] = kern[4 * nnz * ci + 4 * prefix[src] + co * rnnz[src] + k_in_row]

    XT = X.reshape(B, IC, N)
    deg = np.bincount(dst, minlength=N)
    degmax = int(deg.max())

    # order edges by dest, then build ELL slot table
    order = np.argsort(dst, kind="stable")
    e_dst, e_src, e_w = dst[order], src[order], wedge[order]
    dstart = np.concatenate([[0], np.cumsum(np.bincount(e_dst, minlength=N))])

    ovf = max(1, int(((deg > S).reshape(NCORES, JPC)).sum(axis=1).max()))

    in_maps = []
    perms = []
    for k in range(NCORES):
        jglob = np.arange(k * JPC, (k + 1) * JPC)
        # overflow nodes first so the device's fixed 0..ovf-1 overflow
        # matmuls line up with them
        permj = np.argsort(deg[jglob] <= S, kind="stable")
        perms.append(permj)
        jsel = jglob[permj]

        src_ell = np.zeros((JPC, degmax), np.int64)
        w_ell = np.zeros((JPC, degmax, IC, CH), np.float32)
        for jl, j in enumerate(jsel):
            a, b_ = dstart[j], dstart[j + 1]
            src_ell[jl, : b_ - a] = e_src[a:b_]
            w_ell[jl, : b_ - a] = e_w[a:b_]

        def pack(slot_lo, slot_hi, nodes):
            ns = slot_hi - slot_lo
            se = src_ell[nodes, slot_lo:slot_hi]             # [nj, ns]
            xa = XT[:, :, se]                                # [B, IC, nj, ns]
            xg = np.zeros((ns * IC, len(nodes), B), np.float16)
            xg[: ns * IC] = (
                xa.transpose(3, 1, 2, 0).reshape(ns * IC, len(nodes), B)
            )
            wa = w_ell[nodes, slot_lo:slot_hi]               # [nj, ns, IC, CH]
            wg = wa.transpose(1, 2, 0, 3).reshape(ns * IC, len(nodes), CH)
            return xg, wg.astype(np.float16)

        xg_main, wm_main = pack(0, S, np.arange(JPC))
        xg128 = np.zeros((128, JPC, B), np.float16)
        xg128[: S * IC] = xg_main
        wm128 = np.zeros((128, JPC, CH), np.float16)
        wm128[: S * IC] = wm_main

        # overflow block: slots S..degmax for the first `ovf` nodes
        oxg = np.zeros((128, ovf, B), np.float16)
        owm = np.zeros((128, ovf, CH), np.float16)
        nov = (degmax - S) * IC
        if degmax > S:
            xg_o, wm_o = pack(S, degmax, np.arange(ovf))
            oxg[:nov] = xg_o
            owm[:nov] = wm_o

        # bias in physical layout: out_d[(c,b), (j4,co)] = psum of node
        # jl = 4*j4 + c  ->  bias[co*N + jsel[jl]]
        jl_grid = 4 * (np.arange(JPC // 4)[None, :]) + (np.arange(4)[:, None])
        bia = bvec.reshape(CH, N)[:, jsel[jl_grid]]          # [CH, 4c, 64j4]
        biasn = np.broadcast_to(
            bia.transpose(1, 0, 2)[:, None, :, :], (4, B, CH, JPC // 4)
        )
        biasn = (
            biasn.transpose(0, 1, 3, 2).reshape(128, JPC).astype(np.float32)
        )
        in_maps.append(
            {
                "xg": np.ascontiguousarray(xg128),
                "wm": np.ascontiguousarray(wm128),
                "oxg": np.ascontiguousarray(oxg),
                "owm": np.ascontiguousarray(owm),
                "biasn": np.ascontiguousarray(biasn),
            }
        )
    return in_maps, perms, ovf


def run(packed, trace=False, **kwargs):
    from concourse.bass_utils import run_bass_kernel_spmd

    in_maps, perms, ovf = packed
    nc = build_program(ovf, debug=False)
    res = run_bass_kernel_spmd(
        nc, in_maps, core_ids=list(range(NCORES)), trace=trace, **kwargs
    )
    # undo physical layout: dev[(c,b), (j4,co)] -> out[b, co*N + jsel[4*j4+c]]
    outp = np.empty((B, CH * N), np.float32)
    for k in range(NCORES):
        dev = res.results[k]["out"].reshape(4, B, JPC // 4, CH)
        jsel = np.arange(k * JPC, (k + 1) * JPC)[perms[k]]
        vals = dev.transpose(1, 3, 2, 0).reshape(B, CH, JPC)  # [b, co, j4*4+c]
        jl = (4 * np.arange(JPC // 4)[None, :] + np.arange(4)[:, None])
        cols = jsel[jl.T.reshape(JPC)]                        # j for jl order
        for co in range(CH):
            outp[:, co * N + cols] = vals[:, co, :]
    return outp, res


def kernel(inputs, adjacency, kernel, bias):
    packed = pack_inputs(inputs, adjacency, kernel, bias)
    outp, _ = run(packed, trace=False)
    return outp


# revision 11
# speedup vs baseline: 1.1277x; 1.1277x over previous
"""Trainium2 kernel for nn_MAg_90709709292194 (gnn_message_passing).

Computation: out = inputs @ ker_wt + bias, where ker_wt (8192x8192, ~0.9%
dense) holds the `kernel` values scattered into the nonzero pattern of
tile(adjacency, (4, 4)) in row-major nonzero order.

The dense formulation streams 128 MiB of mostly-zero weights; instead this
kernel exploits the graph structure directly. Mirroring the original TF
layer, everything derivable at build() time (adjacency nonzeros, per-edge
4x4 weight blocks, ELL packing/permutations) is host-side prep; the
per-forward-pass math runs on the NeuronCores.

Per-destination-node ELL formulation, dest-sharded over 8 cores (256 dest
nodes per core):
    out[b, co, j] = sum_s sum_ci X[b, ci, src(j, s)] * w[j, s, ci, co]
Each dest node j becomes ONE tensor-engine matmul with K = 128 = (32
in-degree slots x 4 in-channels): stationary = gathered X columns for j's
neighborhood [128, 32 batch] (fp16), moving = that node's packed edge
weights [128, 4 out-channels]. Nodes round-robin the four 32-wide PE column
groups, so four matmuls run concurrently in the array; in-degree > 32
(max 35 here) spills into a second accumulating matmul from a small
overflow block. PSUM accumulates [128 = 4 groups x 32 batch, 256 = 64
nodes x 4 co]; one DVE pass adds bias, and the result is dumped linearly
with the column permutation undone on host.
"""

import numpy as np

N = 2048        # nodes
IC = 4          # input channels
CH = 4          # output channels
B = 32          # batch
NCORES = 8
JPC = N // NCORES   # 256 dest nodes per core
S = 32              # ELL slots (in-degree capacity per matmul)
NXT = 8             # xg streaming tiles (32 nodes each)

_PROGRAM_CACHE = {}


def build_program(ovf, debug=False):
    key = (int(ovf), bool(debug))
    if key in _PROGRAM_CACHE:
        return _PROGRAM_CACHE[key]

    import concourse.bass as bass
    import concourse.bacc as bacc
    import concourse.mybir as mybir
    import concourse.tile as tile

    f32 = mybir.dt.float32
    f16 = mybir.dt.float16

    nc = bacc.Bacc(
        "TRN2", target_bir_lowering=False, debug=debug, num_devices=NCORES
    )
    # xg: gathered neighborhood features, [128=(s,ci), j, b] fp16
    xg_d = nc.dram_tensor("xg", [128, JPC, B], f16, kind="ExternalInput")
    # wm: packed edge weights, [128=(s,ci), j, co] fp16
    wm_d = nc.dram_tensor("wm", [128, JPC, CH], f16, kind="ExternalInput")
    # overflow blocks for nodes with in-degree > S (always >= 1 entry)
    oxg_d = nc.dram_tensor("oxg", [128, ovf, B], f16, kind="ExternalInput")
    owm_d = nc.dram_tensor("owm", [128, ovf, CH], f16, kind="ExternalInput")
    # bias replicated into the physical psum layout [(c,b), (j4,co)] f32
    bias_d = nc.dram_tensor("biasn", [128, JPC], f32, kind="ExternalInput")
    # raw output dump; host undoes the layout permutation
    out_d = nc.dram_tensor("out", [128, JPC], f32, kind="ExternalOutput")

    with tile.TileContext(nc) as tc:
        with (
            tc.tile_pool(name="const", bufs=1) as const,
            tc.tile_pool(name="xgpool", bufs=6) as xgpool,
            tc.tile_pool(name="psum", bufs=1, space=bass.MemorySpace.PSUM) as psum,
        ):
            wm = const.tile([128, JPC * CH], f16)
            nc.scalar.dma_start(out=wm[:, : JPC * CH // 4], in_=wm_d[:, : JPC // 4, :])
            nc.scalar.dma_start(out=wm[:, JPC * CH // 4 :], in_=wm_d[:, JPC // 4 :, :])
            oxg = const.tile([128, ovf * B], f16)
            nc.scalar.dma_start(out=oxg[:], in_=oxg_d[:])
            owm = const.tile([128, ovf * CH], f16)
            nc.scalar.dma_start(out=owm[:], in_=owm_d[:])
            bsn = const.tile([128, JPC], f32)
            nc.scalar.dma_start(out=bsn[:], in_=bias_d[:])

            jpt = JPC // NXT  # nodes per xg tile
            acc = psum.tile([128, JPC], f32)
            for t4 in range(NXT):
                xgt = xgpool.tile([128, jpt * B], f16, tag="xgt")
                nc.sync.dma_start(
                    out=xgt[:], in_=xg_d[:, t4 * jpt : (t4 + 1) * jpt, :]
                )
                for jj in range(jpt):
                    jl = t4 * jpt + jj
                    c = jl % 4
                    j4 = jl // 4
                    nc.tensor.matmul(
                        acc[32 * c : 32 * (c + 1), 4 * j4 : 4 * (j4 + 1)],
                        xgt[:, B * jj : B * (jj + 1)],
                        wm[:, CH * jl : CH * (jl + 1)],
                        start=True,
                        stop=(jl >= ovf),
                        tile_position=(0, 32 * c),
                        skip_group_check=True,
                    )
                    if jl < ovf:
                        # in-degree overflow: accumulate slots S..degmax
                        # immediately so the PSUM group closes right away
                        nc.tensor.matmul(
                            acc[32 * c : 32 * (c + 1), 4 * j4 : 4 * (j4 + 1)],
                            oxg[:, B * jl : B * (jl + 1)],
                            owm[:, CH * jl : CH * (jl + 1)],
                            start=False,
                            stop=True,
                            tile_position=(0, 32 * c),
                            skip_group_check=True,
                        )
            osb = const.tile([128, JPC], f32)
            h = JPC // 2
            nc.vector.tensor_add(osb[:, :h], acc[:, :h], bsn[:, :h])
            nc.sync.dma_start(out=out_d[:, :h], in_=osb[:, :h])
            nc.vector.tensor_add(osb[:, h:], acc[:, h:], bsn[:, h:])
            nc.sync.dma_start(out=out_d[:, h:], in_=osb[:, h:])

    nc.compile()
    _PROGRAM_CACHE[key] = nc
    return nc


def pack_inputs(inputs, adjacency, kernel, bias):
    """Host-side build()-time graph/weight packing + per-core sharding."""
    X = np.asarray(inputs, dtype=np.float32)
    A = np.asarray(adjacency, dtype=np.float32)
    kern = np.asarray(kernel, dtype=np.float32)
    bvec = np.asarray(bias, dtype=np.float32)

    src, dst = np.nonzero(A)          # edge src -> dst, row-major order
    nnz = src.shape[0]
    rnnz = np.bincount(src, minlength=N).astype(np.int64)
    prefix = np.concatenate([[0], np.cumsum(rnnz)[:-1]])
    k_in_row = np.arange(nnz, dtype=np.int64) - prefix[src]
    # per-edge 4x4 weight block, w_e[ci, co]
    wedge = np.empty((nnz, IC, CH), np.float32)
    for ci in range(IC):
        for co in range(CH):
            wedge[:, ci, co] = kern[4 * nnz * ci + 4 * prefix[src] + co * rnnz[src] + k_in_row]

    XT = X.reshape(B, IC, N)
    deg = np.bincount(dst, minlength=N)
    degmax = int(deg.max())

    # order edges by dest, then build ELL slot table
    order = np.argsort(dst, kind="stable")
    e_dst, e_src, e_w = dst[order], src[order], wedge[order]
    dstart = np.concatenate([[0], np.cumsum(np.bincount(e_dst, minlength=N))])

    ovf = max(1, int(((deg > S).reshape(NCORES, JPC)).sum(axis=1).max()))

    in_maps = []
    perms = []
    for k in range(NCORES):
        jglob = np.arange(k * JPC, (k + 1) * JPC)
        # overflow nodes first so the device's fixed 0..ovf-1 overflow
        # matmuls line up with them
        permj = np.argsort(deg[jglob] <= S, kind="stable")
        perms.append(permj)
        jsel = jglob[permj]

        src_ell = np.zeros((JPC, degmax), np.int64)
        w_ell = np.zeros((JPC, degmax, IC, CH), np.float32)
        for jl, j in enumerate(jsel):
            a, b_ = dstart[j], dstart[j + 1]
            src_ell[jl, : b_ - a] = e_src[a:b_]
            w_ell[jl, : b_ - a] = e_w[a:b_]

        def pack(slot_lo, slot_hi, nodes):
            ns = slot_hi - slot_lo
            se = src_ell[nodes, slot_lo:slot_hi]             # [nj, ns]
            xa = XT[:, :, se]                                # [B, IC, nj, ns]
            xg = np.zeros((ns * IC, len(nodes), B), np.float16)
            xg[: ns * IC] = (
                xa.transpose(3, 1, 2, 0).reshape(ns * IC, len(nodes), B)
            )
            wa = w_ell[nodes, slot_lo:slot_hi]               # [nj, ns, IC, CH]
            wg = wa.transpose(1, 2, 0, 3).reshape(ns * IC, len(nodes), CH)
            return xg, wg.astype(np.float16)

        xg_main, wm_main = pack(0, S, np.arange(JPC))
        xg128 = np.zeros((128, JPC, B), np.float16)
        xg128[: S * IC] = xg_main
        wm128 = np.zeros((128, JPC, CH), np.float16)
        wm128[: S * IC] = wm_main

        # overflow block: slots S..degmax for the first `ovf` nodes
        oxg = np.zeros((128, ovf, B), np.float16)
        owm = np.zeros((128, ovf, CH), np.float16)
        nov = (degmax - S) * IC
        if degmax > S:
            xg_o, wm_o = pack(S, degmax, np.arange(ovf))
            oxg[:nov] = xg_o
            owm[:nov] = wm_o

        # bias in physical layout: out_d[(c,b), (j4,co)] = psum of node
        # jl = 4*j4 + c  ->  bias[co*N + jsel[jl]]
        jl_grid = 4 * (np.arange(JPC // 4)[None, :]) + (np.arange(4)[:, None])
        bia = bvec.reshape(CH, N)[:, jsel[jl_grid]]          # [CH, 4c, 64j4]
        biasn = np.broadcast_to(
            bia.transpose(1, 0, 2)[:, None, :, :], (4, B, CH, JPC // 4)
        )
        biasn = (
            biasn.transpose(0, 1, 3, 2).reshape(128, JPC).astype(np.float32)
        )
        in_maps.append(
            {
                "xg": np.ascontiguousarray(xg128),
                "wm": np.ascontiguousarray(wm128),
                "oxg": np.ascontiguousarray(oxg),
                "owm": np.ascontiguousarray(owm),
                "biasn": np.ascontiguousarray(biasn),
            }
        )
    return in_maps, perms, ovf


def run(packed, trace=False, **kwargs):
    from concourse.bass_utils import run_bass_kernel_spmd

    in_maps, perms, ovf = packed
    nc = build_program(ovf, debug=False)
    res = run_bass_kernel_spmd(
        nc, in_maps, core_ids=list(range(NCORES)), trace=trace, **kwargs
    )
    # undo physical layout: dev[(c,b), (j4,co)] -> out[b, co*N + jsel[4*j4+c]]
    outp = np.empty((B, CH * N), np.float32)
    for k in range(NCORES):
        dev = res.results[k]["out"].reshape(4, B, JPC // 4, CH)
        jsel = np.arange(k * JPC, (k + 1) * JPC)[perms[k]]
        vals = dev.transpose(1, 3, 2, 0).reshape(B, CH, JPC)  # [b, co, j4*4+c]
        jl = (4 * np.arange(JPC // 4)[None, :] + np.arange(4)[:, None])
        cols = jsel[jl.T.reshape(JPC)]                        # j for jl order
        for co in range(CH):
            outp[:, co * N + cols] = vals[:, co, :]
    return outp, res


def kernel(inputs, adjacency, kernel, bias):
    packed = pack_inputs(inputs, adjacency, kernel, bias)
    outp, _ = run(packed, trace=False)
    return outp
